# revision 34
# baseline (speedup 1.0000x reference)
"""Trainium2 Bass kernel for nn_CvtNodeInitializer (GNN message passing).

Reference semantics (per edge e = (head, tail)):
    msg_e   = W_msg @ [rel_e ; node_tokens[head_e]]            # [E, H]
    logit_e = msg_e . attn_vector
    masked segment-softmax over tail segments (mask = node_is_cvt[tail]),
    agg[n]  = sum_e softmax_w_e * msg_e                        # [N, H]
    out     = where(cvt, agg + shared_cvt, node_tokens)

Structure (v3):
  * Edges whose tail is not a cvt node contribute nothing (their logits are
    masked to -inf, and non-cvt output rows are node_tokens verbatim), so
    only cvt-tail edges / cvt nodes are processed on device; passthrough
    rows are assembled host-side.
  * Softmax max-subtraction is dropped (logits are O(1); exp is safe in
    fp32, softmax is shift-invariant).  With u_e = exp(logit_e):
      agg[n] = ( Wr @ R[n] + Wn @ G[n] ) / den[n],
      R[n]   = sum_{e in seg n} u_e * rel_e
      G[n]   = sum_{e in seg n} u_e * node_tokens[head_e]
      den[n] = sum_{e in seg n} u_e
  * cvt nodes are packed into (core, block, row) slots of 128-row blocks by
    a balanced greedy (LPT) assignment on degree, so every block holds an
    ~equal edge count; each block's edges live in `cb` chunks of 128 slots.
  * Host marshals bf16 edge-feature streams enr/enn ([P, nchunk, H] each:
    rel_e and node[head_e] rows in chunk-slot order).  The O(E) softmax
    scalars are rank-1 folds computed exactly on host in fp32 (logit_e =
    rel_e.(a@Wr) + node[head_e].(a@Wn), u = exp(logit), rec = 1/den) and
    ride along as tiny fp32 columns; pad slots carry u = 0.  The O(E*H*H)
    projection / aggregation work all runs on TensorE.
  * Per chunk: one-hot weights ohw[e, n] = (iota==tailrow)*u on DVE; 4
    TensorE matmuls (lhsT = enr/enn 128-col slabs, rhs = ohw) accumulate
    R^T/G^T into 4 PSUM tiles (one accumulation group per PSUM tile --
    interleaved groups in one tile misaccumulate on HW, and indirect-DMA
    gathers take only one index per partition per call, which is why the
    node rows are host-marshaled instead of device-gathered).
  * Block tail: PSUM->SBUF copies split across DVE and ScalarE, agg via 4
    matmuls against [WrT; WnT] slabs, t = agg*rec on ScalarE (AP scale),
    + shared_cvt on DVE, bf16 out (host upcasts).  DMAs are batched over
    GRP-block groups to amortize descriptor-generation overhead.
  * All edge-path data is bf16 (fp32 PSUM accumulation; fp32 u/rec).
"""

import math
import heapq
import sys

import numpy as np

sys.path.insert(0, "/opt/trn_rl_repo")

import concourse.bass as bass
import concourse.tile as tile
from concourse import bacc
from concourse import mybir
from concourse.bass_utils import run_bass_kernel_spmd

P = 128  # SBUF partitions / tile edge
NEG = -1.0e30


# ---------------------------------------------------------------------------
# CPU-side sharding / marshaling
# ---------------------------------------------------------------------------

def _prep_inputs(node_tokens, relation_tokens, W_msg, shared_cvt, attn_vector,
                 edge_index, node_is_cvt, n_cores):
    """Filter to cvt-tail edges, pack cvt nodes into balanced (core, block,
    row) slots, and build per-core chunk-tiled bf16 arrays."""
    import ml_dtypes
    N, H = node_tokens.shape
    f32 = np.float32
    bf16 = ml_dtypes.bfloat16
    fp8 = ml_dtypes.float8_e4m3

    heads = np.asarray(edge_index[0], dtype=np.int64)
    tails = np.asarray(edge_index[1], dtype=np.int64)
    cvt = np.asarray(node_is_cvt) != 0

    keep = cvt[tails]
    k_heads = heads[keep]
    k_tails = tails[keep]
    k_rel = np.nonzero(keep)[0]
    Ek = k_heads.size

    cvt_nodes = np.nonzero(cvt)[0]
    Nc = cvt_nodes.size
    cvt_rank = np.full(N, -1, dtype=np.int64)
    cvt_rank[cvt_nodes] = np.arange(Nc)
    e_rank = cvt_rank[k_tails]                       # 0..Nc-1 per edge
    deg = np.bincount(e_rank, minlength=Nc)

    nb = max(1, math.ceil(Nc / (n_cores * P)))       # blocks per core
    nbins = n_cores * nb

    # LPT: nodes in descending degree -> least-loaded bin with < P nodes
    bin_of = np.empty(Nc, dtype=np.int64)
    row_of = np.empty(Nc, dtype=np.int64)
    counts = np.zeros(nbins, dtype=np.int64)
    heap = [(0, b) for b in range(nbins)]
    heapq.heapify(heap)
    for v in np.argsort(-deg, kind="stable"):
        while True:
            load, b = heapq.heappop(heap)
            if counts[b] < P:
                break
        bin_of[v] = b
        row_of[v] = counts[b]
        counts[b] += 1
        heapq.heappush(heap, (load + int(deg[v]), b))

    loads = np.bincount(bin_of, weights=deg.astype(np.float64),
                        minlength=nbins).astype(np.int64)
    cb = max(1, math.ceil(loads.max() / P))          # chunks per block
    nchunk = nb * cb

    # edge -> (core, chunk, lane) slots: edges grouped by bin, sequential
    e_bin = bin_of[e_rank]
    eorder = np.argsort(e_bin, kind="stable")
    bin_counts = np.bincount(e_bin, minlength=nbins)
    off_in_bin = (np.arange(Ek)
                  - np.repeat(np.cumsum(bin_counts) - bin_counts, bin_counts))
    s_bin = e_bin[eorder]
    core = s_bin // nb
    blk = s_bin % nb
    chunk = blk * cb + off_in_bin // P
    lane = off_in_bin % P

    # full logits precomputed per edge (rank-1 folds of attn into W halves)
    Wr = np.asarray(W_msg[:, :H], dtype=f32)         # [H(out), H(in)]
    Wn = np.asarray(W_msg[:, H:], dtype=f32)
    a = np.asarray(attn_vector, dtype=f32)
    nt32 = np.asarray(node_tokens, dtype=f32)
    l_full = (np.asarray(relation_tokens, dtype=f32) @ (a @ Wr))[k_rel] \
        + (nt32 @ (a @ Wn))[k_heads]                 # [Ek]
    u_full = np.exp(l_full)                          # softmax numerators
    den = np.bincount(e_rank, weights=u_full, minlength=Nc)
    rec_node = (1.0 / np.maximum(den, 1e-30)).astype(f32)   # [Nc]

    rel_b = np.asarray(relation_tokens, dtype=f32).astype(bf16)
    node_b = nt32.astype(bf16)
    tailrow = row_of[e_rank[eorder]].astype(f32)

    per_core = []
    for c in range(n_cores):
        m = core == c
        enr_T = np.zeros((P, nchunk, H), dtype=bf16)
        enn_T = np.zeros((P, nchunk, H), dtype=bf16)
        tailf_T = np.zeros((P, nchunk), dtype=f32)
        ucol_T = np.zeros((P, nchunk), dtype=f32)        # pad slots: u = 0
        enr_T[lane[m], chunk[m]] = rel_b[k_rel[eorder[m]]]
        enn_T[lane[m], chunk[m]] = node_b[k_heads[eorder[m]]]
        tailf_T[lane[m], chunk[m]] = tailrow[m]
        ucol_T[lane[m], chunk[m]] = u_full[eorder[m]]
        reccol_T = np.zeros((P, nb), dtype=f32)
        vs = np.nonzero((bin_of // nb) == c)[0]
        reccol_T[row_of[vs], bin_of[vs] % nb] = rec_node[vs]
        per_core.append(dict(enr=enr_T, enn=enn_T, tailf=tailf_T,
                             ucol=ucol_T, reccol=reccol_T))

    # shared (replicated) arrays ------------------------------------------
    w2T = np.concatenate([Wr.T, Wn.T], axis=0).astype(bf16)  # [2H, H]
    shared = dict(
        w2T=np.ascontiguousarray(w2T),
        sharedr=np.ascontiguousarray(
            np.tile(np.asarray(shared_cvt, dtype=f32).astype(bf16), (P, 1))),
        iota=np.ascontiguousarray(
            np.tile(np.arange(P, dtype=f32).astype(bf16), (P, 1))),
    )
    meta = dict(N=N, H=H, Nc=Nc, nb=nb, cb=cb, nchunk=nchunk)
    maps = dict(cvt_nodes=cvt_nodes, bin_of=bin_of, row_of=row_of)
    return per_core, shared, meta, maps


# ---------------------------------------------------------------------------
# Bass kernel builder (SPMD program; per-core data differs, program identical)
# ---------------------------------------------------------------------------

def _build(meta, unroll=1):
    H = meta["H"]
    nb = meta["nb"]
    cb = meta["cb"]
    nchunk = meta["nchunk"]
    f32 = mybir.dt.float32
    bf16 = mybir.dt.bfloat16

    nc = bacc.Bacc("TRN2", target_bir_lowering=False, debug=False)

    enr = nc.declare_dram_parameter("enr", [P, nchunk, H], bf16, isOutput=False)
    enn = nc.declare_dram_parameter("enn", [P, nchunk, H], bf16, isOutput=False)
    tailf = nc.declare_dram_parameter("tailf", [P, nchunk], f32, isOutput=False)
    ucol = nc.declare_dram_parameter("ucol", [P, nchunk], f32, isOutput=False)
    reccol = nc.declare_dram_parameter("reccol", [P, nb], f32, isOutput=False)
    w2T = nc.declare_dram_parameter("w2T", [2 * H, H], bf16, isOutput=False)
    sharedr = nc.declare_dram_parameter("sharedr", [P, H], bf16, isOutput=False)
    iota = nc.declare_dram_parameter("iota", [P, P], bf16, isOutput=False)
    outp = nc.declare_dram_parameter("out", [P, nb, H], bf16, isOutput=True)

    with tile.TileContext(nc) as tc:
        with (
            tc.tile_pool(name="consts", bufs=1) as consts,
            tc.tile_pool(name="edges", bufs=6) as edges,
            tc.tile_pool(name="work", bufs=4) as work,
            tc.tile_pool(name="smalls", bufs=8) as smalls,
            tc.tile_pool(name="blocksb", bufs=3) as blocksb,
            tc.tile_pool(name="ps_seg", bufs=1, space="PSUM") as ps_seg,
            tc.tile_pool(name="ps_agg", bufs=2, space="PSUM") as ps_agg,
        ):
            # ---- constants resident in SBUF -------------------------------
            # ordering matters: the chunk pipeline needs iota/tailf/ucol
            # immediately; w2/sharedr/reccol only at the first block tail.
            iota_sb = consts.tile([P, P], bf16)
            nc.sync.dma_start(out=iota_sb[:], in_=iota[:])
            tailf_sb = consts.tile([P, nchunk], f32)
            nc.sync.dma_start(out=tailf_sb[:], in_=tailf[:])
            ucol_sb = consts.tile([P, nchunk], f32)
            nc.sync.dma_start(out=ucol_sb[:], in_=ucol[:])
            ones_col = consts.tile([P, 1], bf16)
            nc.vector.memset(ones_col[:], 1.0)
            w2_sb = []
            for k in range(4):
                t = consts.tile([P, H], bf16, tag=f"w2_{k}")
                w2_sb.append(t)
            sharedr_sb = consts.tile([P, H], bf16)
            reccol_sb = consts.tile([P, nb], f32)

            def load_tail_consts():
                for k in range(4):
                    nc.sync.dma_start(out=w2_sb[k][:],
                                      in_=w2T[k * P:(k + 1) * P, :])
                nc.sync.dma_start(out=sharedr_sb[:], in_=sharedr[:])
                nc.sync.dma_start(out=reccol_sb[:], in_=reccol[:])

            import contextlib
            loop_ctx = (tc.For_i(0, unroll) if unroll > 1
                        else contextlib.nullcontext())
            GRP = 2                 # blocks per DMA batch
            with loop_ctx:
                enr_sb = enn_sb = out_sb = None
                for b in range(nb):
                    c0 = b * cb
                    if b % GRP == 0:
                        g = min(GRP, nb - b)
                        g0 = b * cb
                        enr_sb = edges.tile([P, GRP * cb, H], bf16, tag="enr")
                        enn_sb = edges.tile([P, GRP * cb, H], bf16, tag="enn")
                        nc.sync.dma_start(out=enr_sb[:, 0:g * cb, :],
                                          in_=enr[:, g0:g0 + g * cb, :])
                        nc.sync.dma_start(out=enn_sb[:, 0:g * cb, :],
                                          in_=enn[:, g0:g0 + g * cb, :])
                        if b == 0:
                            load_tail_consts()
                        out_sb = blocksb.tile([P, GRP, H], bf16, tag="t")
                    boff = (b % GRP) * cb

                    # one accumulation group per PSUM tile
                    seg = []
                    for k in range(4):
                        seg_k = ps_seg.tile([P, P], f32, tag=f"seg{k}",
                                            space="PSUM")
                        seg.append(seg_k)
                    for j in range(cb):
                        gc = c0 + j
                        # ohw[e, n] = (iota == tailrow) * u  (u host-computed)
                        ohw = work.tile([P, P], bf16, tag="ohw")
                        nc.vector.tensor_scalar(
                            out=ohw[:], in0=iota_sb[:],
                            scalar1=tailf_sb[:, gc:gc + 1],
                            scalar2=ucol_sb[:, gc:gc + 1],
                            op0=mybir.AluOpType.is_equal,
                            op1=mybir.AluOpType.mult)
                        st = (j == 0)
                        sp = (j == cb - 1)
                        jj = boff + j
                        nc.tensor.matmul(seg[0][:], lhsT=enr_sb[:, jj, 0:P],
                                         rhs=ohw[:], start=st, stop=sp)
                        nc.tensor.matmul(seg[1][:], lhsT=enr_sb[:, jj, P:H],
                                         rhs=ohw[:], start=st, stop=sp)
                        nc.tensor.matmul(seg[2][:], lhsT=enn_sb[:, jj, 0:P],
                                         rhs=ohw[:], start=st, stop=sp)
                        nc.tensor.matmul(seg[3][:], lhsT=enn_sb[:, jj, P:H],
                                         rhs=ohw[:], start=st, stop=sp)

                    # ---- block tail: agg = (R @ WrT + G @ WnT) / den ------
                    # PSUM->SBUF copies split across DVE and ScalarE
                    rg_sb = blocksb.tile([P, 4 * P], bf16, tag="rg")
                    nc.vector.tensor_copy(out=rg_sb[:, 0:P], in_=seg[0][:])
                    nc.vector.tensor_copy(out=rg_sb[:, P:2 * P], in_=seg[1][:])
                    nc.scalar.activation(
                        out=rg_sb[:, 2 * P:3 * P], in_=seg[2][:],
                        func=mybir.ActivationFunctionType.Copy)
                    nc.scalar.activation(
                        out=rg_sb[:, 3 * P:4 * P], in_=seg[3][:],
                        func=mybir.ActivationFunctionType.Copy)
                    agg_ps = ps_agg.tile([P, H], f32, tag="agg", space="PSUM")
                    for k in range(4):
                        nc.tensor.matmul(
                            agg_ps[:], lhsT=rg_sb[:, k * P:(k + 1) * P],
                            rhs=w2_sb[k][:], start=(k == 0), stop=(k == 3))

                    t_slice = out_sb[:, b % GRP, :]
                    nc.scalar.activation(
                        out=t_slice, in_=agg_ps[:],
                        func=mybir.ActivationFunctionType.Copy,
                        scale=reccol_sb[:, b:b + 1])
                    nc.vector.tensor_add(out=t_slice, in0=t_slice,
                                         in1=sharedr_sb[:])
                    if b % GRP == GRP - 1 or b == nb - 1:
                        gb = (b // GRP) * GRP
                        g = b - gb + 1
                        nc.sync.dma_start(out=outp[:, gb:gb + g, :],
                                          in_=out_sb[:, 0:g, :])

    nc.compile()
    return nc


# ---------------------------------------------------------------------------
# public entry point
# ---------------------------------------------------------------------------

def kernel(node_tokens, relation_tokens, W_msg, shared_cvt, attn_vector,
           edge_index, node_is_cvt):
    node_tokens = np.asarray(node_tokens, dtype=np.float32)
    relation_tokens = np.asarray(relation_tokens, dtype=np.float32)
    W_msg = np.asarray(W_msg, dtype=np.float32)
    shared_cvt = np.asarray(shared_cvt, dtype=np.float32)
    attn_vector = np.asarray(attn_vector, dtype=np.float32)
    edge_index = np.asarray(edge_index)
    node_is_cvt_np = np.asarray(node_is_cvt)

    n_cores = 8
    per_core, shared, meta, maps = _prep_inputs(
        node_tokens, relation_tokens, W_msg, shared_cvt, attn_vector,
        edge_index, node_is_cvt_np, n_cores)

    nc = _build(meta)

    in_maps = []
    for c in range(n_cores):
        m = dict(per_core[c])
        m.update(shared)
        in_maps.append(m)

    res = run_bass_kernel_spmd(nc, in_maps, list(range(n_cores)))

    nb = meta["nb"]
    cvt_nodes = maps["cvt_nodes"]
    bin_of, row_of = maps["bin_of"], maps["row_of"]
    out = node_tokens.copy()
    for c in range(n_cores):
        o = np.asarray(res.results[c]["out"]).astype(np.float32)  # [P, nb, H]
        vs = np.nonzero((bin_of // nb) == c)[0]
        out[cvt_nodes[vs]] = o[row_of[vs], bin_of[vs] % nb, :]
    return out


if __name__ == "__main__":
    pass


# revision 37
# speedup vs baseline: 1.3564x; 1.3564x over previous
"""Trainium2 Bass kernel for nn_CvtNodeInitializer (GNN message passing).

Reference semantics (per edge e = (head, tail)):
    msg_e   = W_msg @ [rel_e ; node_tokens[head_e]]            # [E, H]
    logit_e = msg_e . attn_vector
    masked segment-softmax over tail segments (mask = node_is_cvt[tail]),
    agg[n]  = sum_e softmax_w_e * msg_e                        # [N, H]
    out     = where(cvt, agg + shared_cvt, node_tokens)

Structure (v3):
  * Edges whose tail is not a cvt node contribute nothing (their logits are
    masked to -inf, and non-cvt output rows are node_tokens verbatim), so
    only cvt-tail edges / cvt nodes are processed on device; passthrough
    rows are assembled host-side.
  * Softmax max-subtraction is dropped (logits are O(1); exp is safe in
    fp32, softmax is shift-invariant).  With u_e = exp(logit_e):
      agg[n] = ( Wr @ R[n] + Wn @ G[n] ) / den[n],
      R[n]   = sum_{e in seg n} u_e * rel_e
      G[n]   = sum_{e in seg n} u_e * node_tokens[head_e]
      den[n] = sum_{e in seg n} u_e
  * cvt nodes are packed into (core, block, row) slots of 128-row blocks by
    a balanced greedy (LPT) assignment on degree, so every block holds an
    ~equal edge count; each block's edges live in `cb` chunks of 128 slots.
  * Host marshals bf16 edge-feature streams enr/enn ([P, nchunk, H] each:
    rel_e and node[head_e] rows in chunk-slot order).  The O(E) softmax
    scalars are rank-1 folds computed exactly on host in fp32 (logit_e =
    rel_e.(a@Wr) + node[head_e].(a@Wn), u = exp(logit), rec = 1/den) and
    ride along as tiny fp32 columns; pad slots carry u = 0.  The O(E*H*H)
    projection / aggregation work all runs on TensorE.
  * Per chunk: one-hot weights ohw[e, n] = (iota==tailrow)*u on DVE; 4
    TensorE matmuls (lhsT = enr/enn 128-col slabs, rhs = ohw) accumulate
    R^T/G^T into 4 PSUM tiles (one accumulation group per PSUM tile --
    interleaved groups in one tile misaccumulate on HW, and indirect-DMA
    gathers take only one index per partition per call, which is why the
    node rows are host-marshaled instead of device-gathered).
  * Block tail: PSUM->SBUF copies split across DVE and ScalarE, agg via 4
    matmuls against [WrT; WnT] slabs, t = agg*rec on ScalarE (AP scale),
    + shared_cvt on DVE, bf16 out (host upcasts).  DMAs are batched over
    GRP-block groups to amortize descriptor-generation overhead.
  * The node-row stream is bf16; the rel stream is fp8e4m3 (its error is
    averaged down by the 256-wide Wr contraction); fp32 PSUM accumulation
    and exact fp32 u/rec keep the softmax weights precise.  Measured
    rel_err vs the fp32 reference: 1.6e-2 (gate 2e-2; all-bf16 gives
    7.7e-3 if more margin is ever needed).
"""

import math
import heapq
import sys

import numpy as np

sys.path.insert(0, "/opt/trn_rl_repo")

import concourse.bass as bass
import concourse.tile as tile
from concourse import bacc
from concourse import mybir
from concourse.bass_utils import run_bass_kernel_spmd

P = 128  # SBUF partitions / tile edge
NEG = -1.0e30


# ---------------------------------------------------------------------------
# CPU-side sharding / marshaling
# ---------------------------------------------------------------------------

def _prep_inputs(node_tokens, relation_tokens, W_msg, shared_cvt, attn_vector,
                 edge_index, node_is_cvt, n_cores):
    """Filter to cvt-tail edges, pack cvt nodes into balanced (core, block,
    row) slots, and build per-core chunk-tiled bf16 arrays."""
    import ml_dtypes
    N, H = node_tokens.shape
    f32 = np.float32
    bf16 = ml_dtypes.bfloat16
    fp8 = ml_dtypes.float8_e4m3

    heads = np.asarray(edge_index[0], dtype=np.int64)
    tails = np.asarray(edge_index[1], dtype=np.int64)
    cvt = np.asarray(node_is_cvt) != 0

    keep = cvt[tails]
    k_heads = heads[keep]
    k_tails = tails[keep]
    k_rel = np.nonzero(keep)[0]
    Ek = k_heads.size

    cvt_nodes = np.nonzero(cvt)[0]
    Nc = cvt_nodes.size
    cvt_rank = np.full(N, -1, dtype=np.int64)
    cvt_rank[cvt_nodes] = np.arange(Nc)
    e_rank = cvt_rank[k_tails]                       # 0..Nc-1 per edge
    deg = np.bincount(e_rank, minlength=Nc)

    nb = max(1, math.ceil(Nc / (n_cores * P)))       # blocks per core
    nbins = n_cores * nb

    # LPT: nodes in descending degree -> least-loaded bin with < P nodes
    bin_of = np.empty(Nc, dtype=np.int64)
    row_of = np.empty(Nc, dtype=np.int64)
    counts = np.zeros(nbins, dtype=np.int64)
    heap = [(0, b) for b in range(nbins)]
    heapq.heapify(heap)
    for v in np.argsort(-deg, kind="stable"):
        while True:
            load, b = heapq.heappop(heap)
            if counts[b] < P:
                break
        bin_of[v] = b
        row_of[v] = counts[b]
        counts[b] += 1
        heapq.heappush(heap, (load + int(deg[v]), b))

    loads = np.bincount(bin_of, weights=deg.astype(np.float64),
                        minlength=nbins).astype(np.int64)
    cb = max(1, math.ceil(loads.max() / P))          # chunks per block
    nchunk = nb * cb

    # edge -> (core, chunk, lane) slots: edges grouped by bin, sequential
    e_bin = bin_of[e_rank]
    eorder = np.argsort(e_bin, kind="stable")
    bin_counts = np.bincount(e_bin, minlength=nbins)
    off_in_bin = (np.arange(Ek)
                  - np.repeat(np.cumsum(bin_counts) - bin_counts, bin_counts))
    s_bin = e_bin[eorder]
    core = s_bin // nb
    blk = s_bin % nb
    chunk = blk * cb + off_in_bin // P
    lane = off_in_bin % P

    # full logits precomputed per edge (rank-1 folds of attn into W halves)
    Wr = np.asarray(W_msg[:, :H], dtype=f32)         # [H(out), H(in)]
    Wn = np.asarray(W_msg[:, H:], dtype=f32)
    a = np.asarray(attn_vector, dtype=f32)
    nt32 = np.asarray(node_tokens, dtype=f32)
    l_full = (np.asarray(relation_tokens, dtype=f32) @ (a @ Wr))[k_rel] \
        + (nt32 @ (a @ Wn))[k_heads]                 # [Ek]
    u_full = np.exp(l_full)                          # softmax numerators
    den = np.bincount(e_rank, weights=u_full, minlength=Nc)
    rec_node = np.where(den > 0, 1.0 / np.maximum(den, 1e-300), 0.0) \
        .astype(f32)                                 # [Nc]; deg-0 -> shared only

    rel_b = np.asarray(relation_tokens, dtype=f32).astype(fp8)
    node_b = nt32.astype(bf16)
    tailrow = row_of[e_rank[eorder]].astype(f32)

    per_core = []
    for c in range(n_cores):
        m = core == c
        enr_T = np.zeros((P, nchunk, H), dtype=fp8)
        enn_T = np.zeros((P, nchunk, H), dtype=bf16)
        tailf_T = np.zeros((P, nchunk), dtype=f32)
        ucol_T = np.zeros((P, nchunk), dtype=f32)        # pad slots: u = 0
        enr_T[lane[m], chunk[m]] = rel_b[k_rel[eorder[m]]]
        enn_T[lane[m], chunk[m]] = node_b[k_heads[eorder[m]]]
        tailf_T[lane[m], chunk[m]] = tailrow[m]
        ucol_T[lane[m], chunk[m]] = u_full[eorder[m]]
        reccol_T = np.zeros((P, nb), dtype=f32)
        vs = np.nonzero((bin_of // nb) == c)[0]
        reccol_T[row_of[vs], bin_of[vs] % nb] = rec_node[vs]
        per_core.append(dict(enr=enr_T, enn=enn_T, tailf=tailf_T,
                             ucol=ucol_T, reccol=reccol_T))

    # shared (replicated) arrays ------------------------------------------
    w2T = np.concatenate([Wr.T, Wn.T], axis=0).astype(bf16)  # [2H, H]
    shared = dict(
        w2T=np.ascontiguousarray(w2T),
        sharedr=np.ascontiguousarray(
            np.tile(np.asarray(shared_cvt, dtype=f32).astype(bf16), (P, 1))),
        iota=np.ascontiguousarray(
            np.tile(np.arange(P, dtype=f32).astype(bf16), (P, 1))),
    )
    meta = dict(N=N, H=H, Nc=Nc, nb=nb, cb=cb, nchunk=nchunk)
    maps = dict(cvt_nodes=cvt_nodes, bin_of=bin_of, row_of=row_of)
    return per_core, shared, meta, maps


# ---------------------------------------------------------------------------
# Bass kernel builder (SPMD program; per-core data differs, program identical)
# ---------------------------------------------------------------------------

def _build(meta, unroll=1, grp=4, ebufs=4):
    H = meta["H"]
    nb = meta["nb"]
    cb = meta["cb"]
    nchunk = meta["nchunk"]
    f32 = mybir.dt.float32
    bf16 = mybir.dt.bfloat16

    nc = bacc.Bacc("TRN2", target_bir_lowering=False, debug=False)

    fp8 = mybir.dt.float8e4
    enr = nc.declare_dram_parameter("enr", [P, nchunk, H], fp8, isOutput=False)
    enn = nc.declare_dram_parameter("enn", [P, nchunk, H], bf16, isOutput=False)
    tailf = nc.declare_dram_parameter("tailf", [P, nchunk], f32, isOutput=False)
    ucol = nc.declare_dram_parameter("ucol", [P, nchunk], f32, isOutput=False)
    reccol = nc.declare_dram_parameter("reccol", [P, nb], f32, isOutput=False)
    w2T = nc.declare_dram_parameter("w2T", [2 * H, H], bf16, isOutput=False)
    sharedr = nc.declare_dram_parameter("sharedr", [P, H], bf16, isOutput=False)
    iota = nc.declare_dram_parameter("iota", [P, P], bf16, isOutput=False)
    outp = nc.declare_dram_parameter("out", [P, nb, H], bf16, isOutput=True)

    with tile.TileContext(nc) as tc:
        with (
            tc.tile_pool(name="consts", bufs=1) as consts,
            tc.tile_pool(name="edges", bufs=ebufs) as edges,
            tc.tile_pool(name="work", bufs=4) as work,
            tc.tile_pool(name="smalls", bufs=8) as smalls,
            tc.tile_pool(name="blocksb", bufs=3) as blocksb,
            tc.tile_pool(name="ps_seg", bufs=1, space="PSUM") as ps_seg,
            tc.tile_pool(name="ps_agg", bufs=2, space="PSUM") as ps_agg,
        ):
            # ---- constants resident in SBUF -------------------------------
            # ordering matters: the chunk pipeline needs iota/tailf/ucol
            # immediately; w2/sharedr/reccol only at the first block tail.
            iota_sb = consts.tile([P, P], bf16)
            nc.sync.dma_start(out=iota_sb[:], in_=iota[:])
            tailf_sb = consts.tile([P, nchunk], f32)
            nc.sync.dma_start(out=tailf_sb[:], in_=tailf[:])
            ucol_sb = consts.tile([P, nchunk], f32)
            nc.sync.dma_start(out=ucol_sb[:], in_=ucol[:])
            ones_col = consts.tile([P, 1], bf16)
            nc.vector.memset(ones_col[:], 1.0)
            w2_sb = []
            for k in range(4):
                t = consts.tile([P, H], bf16, tag=f"w2_{k}")
                w2_sb.append(t)
            sharedr_sb = consts.tile([P, H], bf16)
            reccol_sb = consts.tile([P, nb], f32)

            def load_tail_consts():
                for k in range(4):
                    nc.sync.dma_start(out=w2_sb[k][:],
                                      in_=w2T[k * P:(k + 1) * P, :])
                nc.sync.dma_start(out=sharedr_sb[:], in_=sharedr[:])
                nc.sync.dma_start(out=reccol_sb[:], in_=reccol[:])

            import contextlib
            loop_ctx = (tc.For_i(0, unroll) if unroll > 1
                        else contextlib.nullcontext())
            GRP = grp               # blocks per DMA batch
            with loop_ctx:
                enr_sb = enn_sb = out_sb = None
                for b in range(nb):
                    c0 = b * cb
                    if b % GRP == 0:
                        g = min(GRP, nb - b)
                        g0 = b * cb
                        enr_sb = edges.tile([P, GRP * cb, H], fp8, tag="enr")
                        enn_sb = edges.tile([P, GRP * cb, H], bf16, tag="enn")
                        nc.sync.dma_start(out=enr_sb[:, 0:g * cb, :],
                                          in_=enr[:, g0:g0 + g * cb, :])
                        nc.sync.dma_start(out=enn_sb[:, 0:g * cb, :],
                                          in_=enn[:, g0:g0 + g * cb, :])
                        if b == 0:
                            load_tail_consts()
                        out_sb = blocksb.tile([P, GRP, H], bf16, tag="t")
                    boff = (b % GRP) * cb

                    # one accumulation group per PSUM tile
                    seg = []
                    for k in range(4):
                        seg_k = ps_seg.tile([P, P], f32, tag=f"seg{k}",
                                            space="PSUM")
                        seg.append(seg_k)
                    for j in range(cb):
                        gc = c0 + j
                        # ohw[e, n] = (iota == tailrow) * u  (u host-computed)
                        ohw = work.tile([P, P], bf16, tag="ohw")
                        nc.vector.tensor_scalar(
                            out=ohw[:], in0=iota_sb[:],
                            scalar1=tailf_sb[:, gc:gc + 1],
                            scalar2=ucol_sb[:, gc:gc + 1],
                            op0=mybir.AluOpType.is_equal,
                            op1=mybir.AluOpType.mult)
                        st = (j == 0)
                        sp = (j == cb - 1)
                        jj = boff + j
                        nc.tensor.matmul(seg[0][:], lhsT=enr_sb[:, jj, 0:P],
                                         rhs=ohw[:], start=st, stop=sp)
                        nc.tensor.matmul(seg[1][:], lhsT=enr_sb[:, jj, P:H],
                                         rhs=ohw[:], start=st, stop=sp)
                        nc.tensor.matmul(seg[2][:], lhsT=enn_sb[:, jj, 0:P],
                                         rhs=ohw[:], start=st, stop=sp)
                        nc.tensor.matmul(seg[3][:], lhsT=enn_sb[:, jj, P:H],
                                         rhs=ohw[:], start=st, stop=sp)

                    # ---- block tail: agg = (R @ WrT + G @ WnT) / den ------
                    # PSUM->SBUF copies split across DVE and ScalarE
                    rg_sb = blocksb.tile([P, 4 * P], bf16, tag="rg")
                    nc.vector.tensor_copy(out=rg_sb[:, 0:P], in_=seg[0][:])
                    nc.vector.tensor_copy(out=rg_sb[:, P:2 * P], in_=seg[1][:])
                    nc.scalar.activation(
                        out=rg_sb[:, 2 * P:3 * P], in_=seg[2][:],
                        func=mybir.ActivationFunctionType.Copy)
                    nc.scalar.activation(
                        out=rg_sb[:, 3 * P:4 * P], in_=seg[3][:],
                        func=mybir.ActivationFunctionType.Copy)
                    agg_ps = ps_agg.tile([P, H], f32, tag="agg", space="PSUM")
                    for k in range(4):
                        nc.tensor.matmul(
                            agg_ps[:], lhsT=rg_sb[:, k * P:(k + 1) * P],
                            rhs=w2_sb[k][:], start=(k == 0), stop=(k == 3))

                    t_slice = out_sb[:, b % GRP, :]
                    nc.scalar.activation(
                        out=t_slice, in_=agg_ps[:],
                        func=mybir.ActivationFunctionType.Copy,
                        scale=reccol_sb[:, b:b + 1])
                    nc.vector.tensor_add(out=t_slice, in0=t_slice,
                                         in1=sharedr_sb[:])
                    if b % GRP == GRP - 1 or b == nb - 1:
                        gb = (b // GRP) * GRP
                        g = b - gb + 1
                        nc.sync.dma_start(out=outp[:, gb:gb + g, :],
                                          in_=out_sb[:, 0:g, :])

    nc.compile()
    return nc


# ---------------------------------------------------------------------------
# public entry point
# ---------------------------------------------------------------------------

def kernel(node_tokens, relation_tokens, W_msg, shared_cvt, attn_vector,
           edge_index, node_is_cvt):
    node_tokens = np.asarray(node_tokens, dtype=np.float32)
    relation_tokens = np.asarray(relation_tokens, dtype=np.float32)
    W_msg = np.asarray(W_msg, dtype=np.float32)
    shared_cvt = np.asarray(shared_cvt, dtype=np.float32)
    attn_vector = np.asarray(attn_vector, dtype=np.float32)
    edge_index = np.asarray(edge_index)
    node_is_cvt_np = np.asarray(node_is_cvt)

    n_cores = 8
    per_core, shared, meta, maps = _prep_inputs(
        node_tokens, relation_tokens, W_msg, shared_cvt, attn_vector,
        edge_index, node_is_cvt_np, n_cores)

    nc = _build(meta)

    in_maps = []
    for c in range(n_cores):
        m = dict(per_core[c])
        m.update(shared)
        in_maps.append(m)

    res = run_bass_kernel_spmd(nc, in_maps, list(range(n_cores)))

    nb = meta["nb"]
    cvt_nodes = maps["cvt_nodes"]
    bin_of, row_of = maps["bin_of"], maps["row_of"]
    out = node_tokens.copy()
    for c in range(n_cores):
        o = np.asarray(res.results[c]["out"]).astype(np.float32)  # [P, nb, H]
        vs = np.nonzero((bin_of // nb) == c)[0]
        out[cvt_nodes[vs]] = o[row_of[vs], bin_of[vs] % nb, :]
    return out


if __name__ == "__main__":
    pass
